# revision 22
# baseline (speedup 1.0000x reference)
"""AttentionFlow Trainium2 Bass kernel (v2).

Math (per batch):
  d = 256; w = [w_c | w_q | w_m]
  sim[t,j] = s_c[t] + s_q[j] + sum_d C[t,d] w_m[d] Q[j,d]   (+b, which cancels)
  attn = softmax_j(sim);  AQ = attn @ Q
  beta = softmax_t(max_j sim);  AC = beta @ C
  out = concat([C, AQ, C*AQ, C*AC], axis=-1)

Sharding: data-parallel over batch B=32 across 8 NeuronCores (4 batches/core).

Design (~90us, vs ~95-108us for the v1 baseline):
  - Permuted t-layout t = 8p + i (partition-major): each batch's whole output
    row-block is contiguous per partition in HBM, so output DMAs use 4KB+
    lines (32KB/partition) instead of ~3k 1-2KB lines.  All per-t math is
    permutation-invariant; only the T-sums (AC, s_tot) mix t and they are
    order-free.
  - Everything is staged in one [128, NT, 4D] tile per batch
    (cols 0:256 = C landed directly by the input DMA, 256:512 aq,
    512:768 o3=C*aq, 768:1024 o4=C*AC); stage pool bufs=4 so C of batch b+1
    prefetches without waiting on out-DMA completions.  Q is dispatched
    before C: it gates the first PE work of a batch.
  - E' = exp(g + s_q + s_c) holds the FULL similarity: s_q enters as the
    ACT bias column of the exp; s_c enters INSIDE the g PSUM accumulation
    via two extra matmuls with rank-1 stationaries (wc_k x ones) streaming
    the same ctT chunks.  Then n[t] = colmax_j E' directly (no exp(s_c)
    row pass, no n-row muls, no N=1 transposes), and attn normalization is
    unchanged (exp(s_c[t]) cancels in U[:,0:256]/U[:,256]).
  - n columns come from GpSimd partition_all_reduce(max) + a diagonal
    extract (DVE mul by identity + reduce_max).  GpSimd runs ONLY
    partition_all_reduce / partition_broadcast: mixing in gpsimd tensor ops
    forces ucode library swaps at ~6-7us each.
  - U = E' @ [Q | 1 | 1] as ONE N=258 fp32r matmul per t-tile (fp32r needs
    even moving/output sizes, hence the doubled ones column); AC = n^T @
    [C | 1 | 1] likewise, so s_tot falls out of the same accumulation.
  - The beta tail of batch b (AC matmuls, s_tot, broadcast, o4, output DMA)
    is deferred into batch b+1's body after its transposes, so the
    exp -> all_reduce -> diag chain (~5us) is covered by independent PE
    work; PE idle gaps > 3.4us would otherwise also re-engage the HAM
    half-clock throttle.  The last batch splits its tail all_reduce into
    halves and its o4+DMA into quarters to shorten the exposed chain.
"""

import numpy as np

import concourse.bass as bass
import concourse.mybir as mybir
import concourse.tile as tile
from concourse import bacc
from concourse import bass_isa
from concourse.bass_utils import run_bass_kernel_spmd
from concourse.masks import make_identity

F32 = mybir.dt.float32
F32R = mybir.dt.float32r
AF = mybir.ActivationFunctionType
ALU = mybir.AluOpType
AX = mybir.AxisListType

B, T, J, D = 32, 1024, 128, 256
NCORES = 8
BPC = B // NCORES      # batches per core
NT = T // 128          # t-tiles per batch
ND = D // 128          # d-tiles
NCH = T // 512         # 512-wide chunks per batch
TPC = 4                # t-tiles per chunk

USE_F32R = True


def _rr(ap):
    """float32r view of an f32 AP (for rounded producers + matmul operands)."""
    return ap.bitcast(F32R) if USE_F32R else ap


def _bcast_row(ap_1d, nparts):
    """DRAM AP [n] -> [nparts, n] with partition stride 0 (DMA broadcast)."""
    return bass.AP(
        tensor=ap_1d.tensor, offset=ap_1d.offset, ap=[[0, nparts]] + list(ap_1d.ap)
    )


def _fbcast(ap_col, n):
    """[128,1] column AP -> [128, n] with free stride 0."""
    return bass.AP(
        tensor=ap_col.tensor, offset=ap_col.offset,
        ap=[list(ap_col.ap)[0], [0, n]],
    )


def _tile_bcast(ap_2d, reps):
    """[128, n] AP -> [128, reps, n] with 0-stride middle dim."""
    a = list(ap_2d.ap)
    return bass.AP(
        tensor=ap_2d.tensor, offset=ap_2d.offset,
        ap=[a[0], [0, reps]] + a[1:],
    )


def build_nc(use_f32r=None):
    global USE_F32R
    if use_f32r is not None:
        USE_F32R = use_f32r
    nc = bacc.Bacc()
    ctx_in = nc.declare_dram_parameter("context", [BPC, T, D], F32, isOutput=False)
    qry_in = nc.declare_dram_parameter("query", [BPC, J, D], F32, isOutput=False)
    w_in = nc.declare_dram_parameter("w", [3 * D], F32, isOutput=False)
    out_ext = nc.declare_dram_parameter("out", [BPC, T, 4 * D], F32, isOutput=True)

    with tile.TileContext(nc) as tc:
        _body(tc, ctx_in, qry_in, w_in, out_ext)
    nc.finalize()
    return nc


def _body(tc, ctx_in, qry_in, w_in, out_ext):
    nc = tc.nc
    from contextlib import ExitStack

    with ExitStack() as ctx:
        consts = ctx.enter_context(tc.tile_pool(name="consts", bufs=1))
        stage_p = ctx.enter_context(tc.tile_pool(name="stage", bufs=4))
        big = ctx.enter_context(tc.tile_pool(name="big", bufs=2))
        work = ctx.enter_context(tc.tile_pool(name="work", bufs=2))
        tmp = ctx.enter_context(tc.tile_pool(name="tmp", bufs=1))
        # PSUM budget (8 banks): tr 2 + g 2 + u 3 + ac 1 = 8
        ps_tr = ctx.enter_context(tc.tile_pool(name="ps_tr", bufs=2, space="PSUM"))
        ps_g = ctx.enter_context(tc.tile_pool(name="ps_g", bufs=2, space="PSUM"))
        ps_u = ctx.enter_context(tc.tile_pool(name="ps_u", bufs=3, space="PSUM"))
        ps_ac = ctx.enter_context(tc.tile_pool(name="ps_ac", bufs=1, space="PSUM"))

        loads = {}

        def load_batch(bb, nsplit):
            st = stage_p.tile([128, NT, 4 * D], F32, tag="stage")
            qt = work.tile([128, D], F32, tag="Q")
            # Q first: it gates the first PE work (Q^T transposes)
            nc.sync.dma_start(out=qt, in_=qry_in[bb])
            src = ctx_in[bb].rearrange("(p i) d -> p i d", i=NT)
            step = NT // nsplit
            for s in range(nsplit):
                nc.sync.dma_start(
                    out=st[:, s * step : (s + 1) * step, 0:D],
                    in_=src[:, s * step : (s + 1) * step, :],
                )
            loads[bb] = (st, qt)

        # batch-0 input DMAs dispatched before all the consts traffic (v7)
        load_batch(0, 4)

        # --- constants (identity first: it gates the first PE transposes) ---
        ident = consts.tile([128, 128], F32)
        make_identity(nc, ident)
        ones_col = consts.tile([128, 1], F32)
        nc.vector.memset(ones_col, 1.0)

        # w_c / w_m as per-partition columns (two d-tiles each)
        wc_raw = consts.tile([128, ND], F32)
        wm_cols = consts.tile([128, ND], F32)
        for k in range(ND):
            nc.gpsimd.dma_start(
                out=wc_raw[:, k : k + 1],
                in_=w_in[k * 128 : (k + 1) * 128].rearrange("(p o) -> p o", o=1),
            )
            nc.gpsimd.dma_start(
                out=wm_cols[:, k : k + 1],
                in_=w_in[2 * D + k * 128 : 2 * D + (k + 1) * 128].rearrange(
                    "(p o) -> p o", o=1
                ),
            )
        # wc replicated along free dim: (wc x ones) as a rank-1 stationary
        # adds s_c[t] to every j row inside the g accumulation (v11)
        wc_rep = consts.tile([128, ND, 128], F32)
        for k in range(ND):
            nc.scalar.copy(
                _rr(wc_rep[:, k, :]), _fbcast(wc_raw[:, k : k + 1], 128)
            )
        # w_q broadcast to all partitions (for s_q = rowsum(Q * w_q))
        wq_b = consts.tile([128, D], F32)
        nc.gpsimd.dma_start(out=wq_b, in_=_bcast_row(w_in[D : 2 * D], 128))

        def beta_tail_head(S):
            """Deferred AC matmuls + s_tot -> ac_row -> broadcast for batch
            S, issued after the NEXT batch's transposes so the n_all chain
            has a full block of PE work as cover (v7)."""
            b, st, n_all, Cr = S
            ac_ps = ps_ac.tile([1, D + 2], F32, tag="ac")
            for ii in range(NT):
                nc.tensor.matmul(
                    ac_ps,
                    lhsT=_rr(n_all[:, ii : ii + 1]),
                    rhs=_rr(Cr[:, ii, :]),
                    start=(ii == 0),
                    stop=(ii == NT - 1),
                )
            r_s = work.tile([1, 1], F32, tag="r_s")
            nc.vector.reciprocal(r_s, ac_ps[0:1, D : D + 1])
            ac_row = work.tile([1, D], F32, tag="ac_row")
            nc.scalar.activation(ac_row, ac_ps[0:1, 0:D], AF.Copy, scale=r_s)
            acb = work.tile([128, D], F32, tag="acb")
            nc.gpsimd.partition_broadcast(acb, ac_row, channels=128)
            return acb

        def beta_tail_finish(S, acb, nsplit=2):
            """o4 = C * AC (DVE) + staged output DMA, split so the first
            part's DMA fires while later parts compute (v8/v12)."""
            b, st = S[0], S[1]
            out_r = out_ext[b].rearrange("(p i) d -> p i d", i=NT)
            h = NT // nsplit
            for s in range(nsplit):
                sl = slice(s * h, (s + 1) * h)
                nc.vector.tensor_mul(
                    st[:, sl, 3 * D : 4 * D], st[:, sl, 0:D], _tile_bcast(acb, h)
                )
                nc.sync.dma_start(out=out_r[:, sl, :], in_=st[:, sl, :])

        prev = None
        for b in range(BPC):
            if b + 1 < BPC:
                load_batch(b + 1, 2)
            st, Q = loads.pop(b)

            # ---- Q^T, with w_m folded in: qwT[d, j] = Q[j, d] * w_m[d] ----
            qt_ps = ps_tr.tile([128, D], F32, tag="tr")
            for k in range(ND):
                nc.tensor.transpose(
                    qt_ps[:, k * 128 : (k + 1) * 128], Q[:, k * 128 : (k + 1) * 128],
                    ident,
                )
            qwT = work.tile([128, D], F32, tag="qwT")
            for k in range(ND):
                nc.scalar.activation(
                    _rr(qwT[:, k * 128 : (k + 1) * 128]),
                    qt_ps[:, k * 128 : (k + 1) * 128],
                    AF.Copy,
                    scale=wm_cols[:, k : k + 1],
                )

            # ---- Qaug = [Q | 1] rounded (rhs of the U matmuls) ----
            Qaug = work.tile([128, D + 2], F32, tag="Qaug")
            nc.scalar.copy(_rr(Qaug[:, 0:D]), Q)
            nc.vector.tensor_copy(
                _rr(Qaug[:, D : D + 2]), _fbcast(ones_col, 2)
            )

            # ---- s_q column: rowsum(Q * w_q) ----
            sq_scr = tmp.tile([128, D], F32, tag="sq_scr")
            sq_col = work.tile([128, 1], F32, tag="sq_col")
            nc.vector.tensor_mul(sq_scr, Q, wq_b)
            nc.vector.reduce_sum(out=sq_col, in_=sq_scr, axis=AX.X)

            # ---- C^T via PE transposes ----
            ctT = big.tile([128, ND, NT, 128], F32, tag="ctT")
            for i2 in range(NT // 2):
                ct_ps = ps_tr.tile([128, 2 * ND * 128], F32, tag="tr")
                for u in range(2):
                    i = 2 * i2 + u
                    for k in range(ND):
                        nc.tensor.transpose(
                            ct_ps[:, (2 * u + k) * 128 : (2 * u + k + 1) * 128],
                            st[:, i, k * 128 : (k + 1) * 128],
                            ident,
                        )
                dst = _rr(ctT[:, :, 2 * i2 : 2 * i2 + 2, :])
                srcv = ct_ps.rearrange("p (t k x) -> p k t x", t=2, k=ND)
                if i2 % 2 == 0:
                    nc.scalar.copy(dst, srcv)
                else:
                    nc.vector.tensor_copy(dst, srcv)

            # ---- previous batch: deferred AC + s_tot/ac_row/bcast (v7) ----
            acb_prev = beta_tail_head(prev) if prev is not None else None

            # ---- per 512-chunk: g + s_c row matmuls ----
            ET = big.tile([128, T], F32, tag="ET")
            n_all = work.tile([128, NT], F32, tag="n_all")
            g_list = []
            for c in range(NCH):
                g_ps = ps_g.tile([128, 512], F32, tag="g")
                for k in range(ND):
                    nc.tensor.matmul(
                        g_ps,
                        lhsT=_rr(qwT[:, k * 128 : (k + 1) * 128]),
                        rhs=_rr(ctT[:, k, TPC * c : TPC * (c + 1), :]),
                        start=(k == 0),
                        stop=False,
                        skip_group_check=True,
                    )
                for k in range(ND):
                    nc.tensor.matmul(
                        g_ps,
                        lhsT=_rr(wc_rep[:, k, :]),
                        rhs=_rr(ctT[:, k, TPC * c : TPC * (c + 1), :]),
                        start=False,
                        stop=(k == ND - 1),
                        skip_group_check=True,
                    )
                g_list.append(g_ps)

            # ---- Cr = [C | 1] rounded (rhs of the AC matmuls) ----
            Cr = big.tile([128, NT, D + 2], F32, tag="Cr")
            nc.vector.tensor_copy(
                _rr(Cr[:, :, D : D + 2]),
                bass.AP(tensor=ones_col.tensor, offset=ones_col.offset,
                        ap=[list(ones_col.ap)[0], [0, NT], [0, 2]]),
            )
            h = NT // 2
            nc.scalar.copy(_rr(Cr[:, 0:h, 0:D]), st[:, 0:h, 0:D])
            nc.vector.tensor_copy(_rr(Cr[:, h:NT, 0:D]), st[:, h:NT, 0:D])

            # ---- per chunk: fold s_c (rank-1), E' = exp, colmax, diag ----
            # o4 + out-DMA of the previous batch are issued between the two
            # chunks so the DVE has work while the GpSimd all_reduce runs.
            for c in range(NCH):
                g_ps = g_list[c]
                nc.scalar.activation(
                    _rr(ET[:, c * 512 : (c + 1) * 512]), g_ps, AF.Exp, bias=sq_col
                )
                # colmax over j via PE transposes of E' tiles + one DVE
                # free-dim reduce_max -- keeps GpSimd off the critical path
                # (its all_reduce ran ~2us per chunk and serialized) (v13)
                tp_ps = ps_tr.tile([128, TPC, 128], F32, tag="tr")
                for i in range(TPC):
                    t0 = (TPC * c + i) * 128
                    nc.tensor.transpose(
                        tp_ps[:, i, :], ET[:, t0 : t0 + 128], ident
                    )
                nc.vector.reduce_max(
                    out=_rr(n_all[:, TPC * c : TPC * (c + 1)]), in_=tp_ps,
                    axis=AX.X,
                )

            # ---- per t-tile: U = E' @ [Q|1]; aq on ACT; o3 on DVE ----
            r_all = work.tile([128, NT], F32, tag="r_all")
            for i in range(NT):
                u_ps = ps_u.tile([128, D + 2], F32, tag="u")
                nc.tensor.matmul(
                    u_ps, lhsT=_rr(ET[:, i * 128 : (i + 1) * 128]), rhs=_rr(Qaug)
                )
                nc.vector.reciprocal(r_all[:, i : i + 1], u_ps[:, D : D + 1])
                nc.scalar.activation(
                    st[:, i, D : 2 * D], u_ps[:, 0:D], AF.Copy,
                    scale=r_all[:, i : i + 1],
                )
                if i % TPC == TPC - 1:
                    # o3 for this chunk's 4 tiles (one DVE pass)
                    j0 = i - (TPC - 1)
                    nc.vector.tensor_mul(
                        st[:, j0 : i + 1, 2 * D : 3 * D],
                        st[:, j0 : i + 1, D : 2 * D],
                        st[:, j0 : i + 1, 0:D],
                    )

            # ---- previous batch: o4 + output DMA (after the U loop so the
            #      DVE recips/aq aren't stuck behind the big o4 pass) ----
            if prev is not None:
                beta_tail_finish(prev, acb_prev)
            prev = (b, st, n_all, Cr)

        beta_tail_finish(prev, beta_tail_head(prev), nsplit=4)


_NC_CACHE = {}


def kernel(context, query, w, b, _trace=False):
    context = np.ascontiguousarray(context, dtype=np.float32)
    query = np.ascontiguousarray(query, dtype=np.float32)
    w = np.ascontiguousarray(w, dtype=np.float32)

    if "nc" not in _NC_CACHE:
        _NC_CACHE["nc"] = build_nc()
    nc = _NC_CACHE["nc"]

    in_maps = [
        {
            "context": context[i * BPC : (i + 1) * BPC],
            "query": query[i * BPC : (i + 1) * BPC],
            "w": w,
        }
        for i in range(NCORES)
    ]
    try:
        res = run_bass_kernel_spmd(
            nc, in_maps, core_ids=list(range(NCORES)), trace=_trace
        )
    except Exception:
        # A previous process may have left the device wedged; reset and retry.
        import ctypes

        import jax

        jax.devices()
        lib = ctypes.CDLL("/opt/axon/libaxon_pjrt.so")
        if hasattr(lib, "axon_reset"):
            lib.axon_reset()
        res = run_bass_kernel_spmd(
            nc, in_maps, core_ids=list(range(NCORES)), trace=_trace
        )
    out = np.concatenate([res.results[i]["out"] for i in range(NCORES)], axis=0)
    if _trace:
        kernel.last_exec_time_ns = res.exec_time_ns
        kernel.last_results = res
    return out


if __name__ == "__main__":
    rng = np.random.default_rng(0)
    inputs = {
        "context": rng.standard_normal((B, T, D), dtype=np.float32),
        "query": rng.standard_normal((B, J, D), dtype=np.float32),
        "w": (rng.standard_normal(3 * D).astype(np.float32) / np.sqrt(3 * D)),
        "b": np.zeros(1, np.float32),
    }
    out = kernel(**inputs)
    print("out", out.shape, out.dtype, float(np.abs(out).mean()))
